# revision 13
# baseline (speedup 1.0000x reference)
"""DeformConvTranspose2d Bass kernel for 8 Trainium2 NeuronCores.

Strategy (data-parallel over batch, one batch element per core):

The op is: per-pixel GEMM (cols = x^T W per tap), modulate by mask, then
bilinear scatter-add into the (stride-2 transposed-conv, offset-shifted)
output grid.

Scatter-adds are hostile to Trainium, so the kernel is restructured into
dense matmuls using host-side (numpy) index preprocessing of the offsets:

1. For each tap k, every input pixel's target output row iy0 = floor(y) is
   known on the host.  Pixels are bucketed by target row (129 buckets,
   capacity C=64, zero-padded; actual max fill is 50) and x is shipped
   pre-permuted into bucket order: x_perm [9, 256, 8320] (bf16).
2. Device GEMM per tap: V[slot, cout] = x_perm^T @ W_k, tiled M=128,
   accumulated over Cin in PSUM (fp32), copied to SBUF as bf16.
3. The bilinear scatter for output row r only involves slots in buckets
   r, r+1 (the two y-corners), i.e. the contiguous slot range
   [64r, 64r+128).  A host-built one-hot matrix OH[r] (x-corner weights *
   y-corner weight * mask folded in) turns the scatter into a matmul:
   out_row[px, cout] += OH_chunk^T @ V_chunk, accumulated over the 9 taps
   in PSUM.  Even rows are one K=128 matmul per tap; odd rows straddle two
   V tiles and use two K=64 matmuls.
4. bias is folded in via a reserved all-ones one-hot row pointing at a
   V slot that is overwritten with the bias vector.

Output is written [OH*OW, 256] fp32 per core and transposed on the host.
"""

import os
import sys

sys.path.insert(0, "/opt/trn_rl_repo")

import numpy as np

from concourse import bass, mybir
import concourse.tile as tile

BF16 = mybir.dt.np(mybir.dt.bfloat16)

# problem constants (hardcoded per contract)
B = 8
CIN = 256
COUT = 256
H = W = 64
NK = 9
KH = KW = 3
STRIDE, PAD, OUT_PAD, DIL = 2, 1, 1, 1
OHH = (H - 1) * STRIDE - 2 * PAD + DIL * (KH - 1) + 1 + OUT_PAD  # 128
OWW = OHH  # 128

C = 64          # bucket capacity (max observed fill is 50)
NBUCKET = 129   # target-row buckets: iy0 in [-1, 127] -> bucket iy0+1 in [0, 128]
SLOTS = NBUCKET * C + C  # 8320: one pad bucket so every 128-slot chunk is in range
NTILE = SLOTS // 128     # 65 V tiles of 128 slots per tap
ROWS_PER_STRIP = 16
NSTRIP = OHH // ROWS_PER_STRIP  # 8
TILES_PER_STRIP = 9     # V tiles [8s, 8s+8] cover scatter rows [16s, 16s+16)


# ---------------------------------------------------------------------------
# Walrus codegen workaround: the TRN2 backend accepts only ONE sync wait per
# instruction.  After Tile lowering, hoist extra waits into standalone
# InstEventSemaphore instructions on the same engine, placed just before the
# original instruction (same-engine program order preserves semantics).
# ---------------------------------------------------------------------------
def _split_multi_waits(nc, max_waits=1):
    n = 0
    for fn in nc.m.functions:
        for bb in fn.blocks:
            out = []
            changed = False
            for inst in bb.instructions:
                si = inst.sync_info
                if si is not None and si.on_wait and len(si.on_wait) > max_waits:
                    waits = list(si.on_wait)
                    for w in waits[:-max_waits]:
                        ev = mybir.InstEventSemaphore(
                            name=f"evsplit-{n}",
                            engine=inst.engine,
                            ins=[],
                            outs=[],
                            sync_info=mybir.SyncInfo(on_wait=[w], on_update=[]),
                        )
                        n += 1
                        nc.register_instruction(ev, overwrite=True)
                        out.append(ev)
                    inst.sync_info = mybir.SyncInfo(
                        on_wait=waits[-max_waits:],
                        on_update=list(si.on_update or []),
                    )
                    changed = True
                out.append(inst)
            if changed:
                bb.instructions = out


# ---------------------------------------------------------------------------
# Host-side preprocessing
# ---------------------------------------------------------------------------
def _prep_core(x_b, offset_b, mask_b):
    """Build x_perm [NK, CIN, SLOTS] and oh [128, 128, NK, 128] for one batch."""
    off = offset_b.reshape(NK, 2, H, W).astype(np.float64)
    m = mask_b.reshape(NK, H * W).astype(np.float64)

    ky = (np.arange(NK) // KW).astype(np.float64) * DIL
    kx = (np.arange(NK) % KW).astype(np.float64) * DIL
    base_y = np.arange(H, dtype=np.float64) * STRIDE - PAD
    base_x = np.arange(W, dtype=np.float64) * STRIDE - PAD

    y = off[:, 0] + base_y[None, :, None] + ky[:, None, None]   # [NK, H, W]
    xp = off[:, 1] + base_x[None, None, :] + kx[:, None, None]

    iy0 = np.floor(y).astype(np.int64).reshape(NK, H * W)
    fy = (y - np.floor(y)).reshape(NK, H * W)
    ix0 = np.floor(xp).astype(np.int64).reshape(NK, H * W)
    fx = (xp - np.floor(xp)).reshape(NK, H * W)

    bi = iy0 + 1
    valid = (bi >= 0) & (bi <= 128)

    x_perm = np.zeros((NK, CIN, SLOTS), dtype=np.float32)
    # even output rows r=2m: the slot chunk [64r, 64r+128) IS V tile m -> one
    # K=128 matmul per tap, one-hot ohe[r//2].
    # odd rows straddle tiles m=(r-1)/2, m+1; HW matmuls must start at
    # partition 0, so use two full K=128 matmuls against each whole tile with
    # out-of-chunk rows zeroed: oho[(r-1)//2, 0] (vs tile m) and
    # oho[(r-1)//2, 1] (vs tile m+1).
    ohe = np.zeros((OHH // 2, 128, NK, OWW), dtype=np.float32)
    oho = np.zeros((OHH // 2, 2, 128, NK, OWW), dtype=np.float32)

    for k in range(NK):
        pxv = np.nonzero(valid[k])[0]
        order = pxv[np.argsort(bi[k, pxv], kind="stable")]
        bsort = bi[k, order]
        # rank within bucket
        start = np.searchsorted(bsort, np.arange(NBUCKET), side="left")
        rank = np.arange(len(order)) - start[bsort]
        fill = np.bincount(bsort, minlength=NBUCKET)
        if fill.max() > C - 2:
            raise RuntimeError(f"bucket overflow: max fill {fill.max()} > {C-2}")
        slot = bsort * C + rank
        x_perm[k][:, slot] = x_b[:, order]

        # corner dy=0 hits row r0=bi-1 with wy=1-fy; corner dy=1 hits row
        # r1=bi with wy=fy.  Local index within the target chunk's V tile(s):
        #   even r, bucket r+1 (dy=0): local 64+rank   | bucket r (dy=1): rank
        #   odd  r: part a = tile m local 64+rank (dy=1 corner, bucket r)
        #           part b = tile m+1 local rank (dy=0 corner, bucket r+1)
        for (r_arr, wy, is_dy0) in (
            (bsort - 1, 1.0 - fy[k, order], True),
            (bsort, fy[k, order], False),
        ):
            rok = (r_arr >= 0) & (r_arr <= OHH - 1)
            even = rok & (r_arr % 2 == 0)
            odd = rok & (r_arr % 2 == 1)
            for (col, wx) in (
                (ix0[k, order], 1.0 - fx[k, order]),
                (ix0[k, order] + 1, fx[k, order]),
            ):
                colok = (col >= 0) & (col <= OWW - 1)
                wgt = m[k, order] * wy * wx
                ce = even & colok
                co = odd & colok
                if is_dy0:
                    ohe[r_arr[ce] // 2, 64 + rank[ce], k, col[ce]] = wgt[ce]
                    oho[(r_arr[co] - 1) // 2, 1, rank[co], k, col[co]] = wgt[co]
                else:
                    ohe[r_arr[ce] // 2, rank[ce], k, col[ce]] = wgt[ce]
                    oho[(r_arr[co] - 1) // 2, 0, 64 + rank[co], k, col[co]] = wgt[co]

    # bias hook: all-ones one-hot row pointing at a pad V slot (rank 63 of
    # bucket r+1) that the device overwrites with the bias vector.
    ohe[:, 127, 0, :] = 1.0   # tile m partition 127 = bucket 2m+1 rank 63
    oho[:, 1, 63, 0, :] = 1.0  # tile m+1 partition 63 = bucket r+1 rank 63
    return x_perm.astype(BF16), ohe.astype(BF16), oho.astype(BF16)


def _prep_all(x, weight, offset, mask, bias):
    w_r = np.ascontiguousarray(
        weight.reshape(CIN, COUT, NK).transpose(0, 2, 1)
    ).astype(BF16)  # [CIN, NK, COUT]
    bias_rep = np.zeros((128, COUT), np.float32)
    bias_rep[63] = bias
    bias_rep[127] = bias
    bias_rep = bias_rep.astype(BF16)
    in_maps = []
    for b in range(B):
        x_perm, ohe, oho = _prep_core(x[b].reshape(CIN, H * W), offset[b], mask[b])
        in_maps.append({"xp": x_perm, "ohe": ohe, "oho": oho, "w": w_r, "br": bias_rep})
    return in_maps


# ---------------------------------------------------------------------------
# Device program
# ---------------------------------------------------------------------------
def build_nc(reps=1):
    nc = bass.Bass()
    xp = nc.dram_tensor("xp", [NK, CIN, SLOTS], mybir.dt.bfloat16, kind="ExternalInput")
    ohe = nc.dram_tensor("ohe", [OHH // 2, 128, NK, OWW], mybir.dt.bfloat16, kind="ExternalInput")
    oho = nc.dram_tensor("oho", [OHH // 2, 2, 128, NK, OWW], mybir.dt.bfloat16, kind="ExternalInput")
    wd = nc.dram_tensor("w", [CIN, NK, COUT], mybir.dt.bfloat16, kind="ExternalInput")
    brd = nc.dram_tensor("br", [128, COUT], mybir.dt.bfloat16, kind="ExternalInput")
    out = nc.dram_tensor("out", [OHH * OWW, COUT], mybir.dt.float32, kind="ExternalOutput")

    with tile.TileContext(nc) as tc:
        with tc.tile_pool(name="const", bufs=1) as cpool, \
             tc.tile_pool(name="xt", bufs=6) as xpool, \
             tc.tile_pool(name="v", bufs=2 * NK * TILES_PER_STRIP + 8) as vpool, \
             tc.tile_pool(name="oht", bufs=6) as ohpool, \
             tc.tile_pool(name="ot", bufs=6) as opool, \
             tc.tile_pool(name="pg", bufs=3, space="PSUM") as pgpool, \
             tc.tile_pool(name="po", bufs=3, space="PSUM") as popool:

            # resident weights [2][128, NK*COUT] and bias [2, COUT]
            wt = []
            for h in range(2):
                t = cpool.tile([128, NK * COUT], mybir.dt.bfloat16, tag=f"w{h}")
                nc.sync.dma_start(out=t[:], in_=wd[h * 128:(h + 1) * 128].rearrange("a b c -> a (b c)"))
                wt.append(t)
            bt = cpool.tile([128, COUT], mybir.dt.bfloat16, tag="bias")
            nc.sync.dma_start(out=bt[:], in_=brd[:])

            for rep in range(reps):
                for s in range(NSTRIP):
                    base_tile = NSTRIP * s  # first V tile index of this strip
                    # ---- GEMM phase: V tiles [base_tile, base_tile+9) per tap
                    vt = {}
                    for k in range(NK):
                        xts = []
                        for h in range(2):
                            t = xpool.tile([128, TILES_PER_STRIP * 128],
                                           mybir.dt.bfloat16, tag=f"x{h}")
                            nc.sync.dma_start(
                                out=t[:],
                                in_=xp[k, h * 128:(h + 1) * 128,
                                       base_tile * 128:(base_tile + TILES_PER_STRIP) * 128],
                            )
                            xts.append(t)
                        for c in range(TILES_PER_STRIP):
                            ps = pgpool.tile([128, COUT], mybir.dt.float32, tag="pg")
                            for h in range(2):
                                nc.tensor.matmul(
                                    out=ps[:],
                                    lhsT=xts[h][:, c * 128:(c + 1) * 128],
                                    rhs=wt[h][:, k * COUT:(k + 1) * COUT],
                                    start=(h == 0),
                                    stop=(h == 1),
                                )
                            v = vpool.tile([128, COUT], mybir.dt.bfloat16, tag="v")
                            if k == 0:
                                # fold bias into the pad slots (partitions
                                # 63/127 are always-zero bucket pad ranks)
                                nc.vector.tensor_add(out=v[:], in0=ps[:], in1=bt[:])
                            elif (k * TILES_PER_STRIP + c) % 2 == 0:
                                nc.vector.tensor_copy(out=v[:], in_=ps[:])
                            else:
                                nc.scalar.copy(out=v[:], in_=ps[:])
                            vt[(k, c)] = v

                    # ---- scatter phase: output rows of this strip
                    for j in range(ROWS_PER_STRIP):
                        r = s * ROWS_PER_STRIP + j
                        if j % 2 == 0:
                            oht = ohpool.tile([128, NK * OWW], mybir.dt.bfloat16, tag="ohe")
                            nc.sync.dma_start(
                                out=oht[:], in_=ohe[r // 2].rearrange("a b c -> a (b c)")
                            )
                            # (oh column offset, V tile local index)
                            mms = [(0, j // 2)]
                        else:
                            oht = ohpool.tile([128, 2 * NK * OWW], mybir.dt.bfloat16, tag="oho")
                            for t in range(2):
                                nc.sync.dma_start(
                                    out=oht[:, t * NK * OWW:(t + 1) * NK * OWW],
                                    in_=oho[(r - 1) // 2, t].rearrange("a b c -> a (b c)"),
                                )
                            mms = [(0, (j - 1) // 2), (NK * OWW, (j - 1) // 2 + 1)]
                        po = popool.tile([128, COUT], mybir.dt.float32, tag="po")
                        chain = [(k, off, cidx) for k in range(NK) for (off, cidx) in mms]
                        for i, (k, off, cidx) in enumerate(chain):
                            nc.tensor.matmul(
                                out=po[:],
                                lhsT=oht[:, off + k * OWW:off + (k + 1) * OWW],
                                rhs=vt[(k, cidx)][:],
                                start=(i == 0),
                                stop=(i == len(chain) - 1),
                            )
                        ot = opool.tile([128, COUT], mybir.dt.float32, tag="ot")
                        if j % 2 == 0:
                            nc.vector.tensor_copy(out=ot[:], in_=po[:])
                        else:
                            nc.scalar.copy(out=ot[:], in_=po[:])
                        nc.sync.dma_start(out=out[r * OWW:(r + 1) * OWW, :], in_=ot[:])
    _split_multi_waits(nc)
    return nc


_CACHED = {}


def _get_nc(reps=1):
    if reps not in _CACHED:
        _CACHED[reps] = build_nc(reps)
    return _CACHED[reps]


def run_on_hw(in_maps, reps=1):
    from concourse.bass_utils import run_bass_kernel_spmd

    nc = _get_nc(reps)
    res = run_bass_kernel_spmd(nc, in_maps, list(range(B)))
    return res


def kernel(x, weight, offset, mask, bias):
    x = np.asarray(x, dtype=np.float32)
    weight = np.asarray(weight, dtype=np.float32)
    offset = np.asarray(offset, dtype=np.float32)
    mask = np.asarray(mask, dtype=np.float32)
    bias = np.asarray(bias, dtype=np.float32)

    in_maps = _prep_all(x, weight, offset, mask, bias)
    res = run_on_hw(in_maps, reps=1)
    out = np.empty((B, COUT, OHH, OWW), dtype=np.float32)
    for b in range(B):
        out[b] = res.results[b]["out"].T.reshape(COUT, OHH, OWW)
    return out


# revision 14
# speedup vs baseline: 2.7672x; 2.7672x over previous
"""DeformConvTranspose2d Bass kernel for 8 Trainium2 NeuronCores.

Strategy (data-parallel over batch, one batch element per core):

The op is: per-pixel GEMM (cols = x^T W per tap), modulate by mask, then
bilinear scatter-add into the (stride-2 transposed-conv, offset-shifted)
output grid.

Scatter-adds are hostile to Trainium, so the kernel is restructured into
dense matmuls using host-side (numpy) index preprocessing of the offsets:

1. For each tap k, every input pixel's target output row iy0 = floor(y) is
   known on the host.  Pixels are bucketed by target row (129 buckets,
   capacity C=64, zero-padded; actual max fill is 50) and x is shipped
   pre-permuted into bucket order: x_perm [9, 256, 8320] (bf16).
2. Device GEMM per tap: V[slot, cout] = x_perm^T @ W_k, tiled M=128,
   accumulated over Cin in PSUM (fp32), copied to SBUF as bf16.
3. The bilinear scatter for output row r only involves slots in buckets
   r, r+1 (the two y-corners), i.e. the contiguous slot range
   [64r, 64r+128).  A host-built one-hot matrix OH[r] (x-corner weights *
   y-corner weight * mask folded in) turns the scatter into a matmul:
   out_row[px, cout] += OH_chunk^T @ V_chunk, accumulated over the 9 taps
   in PSUM.  Even rows are one K=128 matmul per tap; odd rows straddle two
   V tiles and use two K=64 matmuls.
4. bias is folded in via a reserved all-ones one-hot row pointing at a
   V slot that is overwritten with the bias vector.

Output is written [OH*OW, 256] fp32 per core and transposed on the host.
"""

import os
import sys

sys.path.insert(0, "/opt/trn_rl_repo")

import numpy as np

from concourse import bass, mybir
import concourse.tile as tile

BF16 = mybir.dt.np(mybir.dt.bfloat16)

# problem constants (hardcoded per contract)
B = 8
CIN = 256
COUT = 256
H = W = 64
NK = 9
KH = KW = 3
STRIDE, PAD, OUT_PAD, DIL = 2, 1, 1, 1
OHH = (H - 1) * STRIDE - 2 * PAD + DIL * (KH - 1) + 1 + OUT_PAD  # 128
OWW = OHH  # 128

C = 64          # bucket capacity (max observed fill is 50)
NBUCKET = 129   # target-row buckets: iy0 in [-1, 127] -> bucket iy0+1 in [0, 128]
SLOTS = NBUCKET * C + C  # 8320: one pad bucket so every 128-slot chunk is in range
NTILE = SLOTS // 128     # 65 V tiles of 128 slots per tap
ROWS_PER_STRIP = 16
NSTRIP = OHH // ROWS_PER_STRIP  # 8
TILES_PER_STRIP = 9     # V tiles [8s, 8s+8] cover scatter rows [16s, 16s+16)


# ---------------------------------------------------------------------------
# Walrus codegen workaround: the TRN2 backend accepts only ONE sync wait per
# instruction.  After Tile lowering, hoist extra waits into standalone
# InstEventSemaphore instructions on the same engine, placed just before the
# original instruction (same-engine program order preserves semantics).
# ---------------------------------------------------------------------------
def _split_multi_waits(nc, max_waits=1):
    n = 0
    for fn in nc.m.functions:
        for bb in fn.blocks:
            out = []
            changed = False
            for inst in bb.instructions:
                si = inst.sync_info
                if si is not None and si.on_wait and len(si.on_wait) > max_waits:
                    waits = list(si.on_wait)
                    for w in waits[:-max_waits]:
                        ev = mybir.InstEventSemaphore(
                            name=f"evsplit-{n}",
                            engine=inst.engine,
                            ins=[],
                            outs=[],
                            sync_info=mybir.SyncInfo(on_wait=[w], on_update=[]),
                        )
                        n += 1
                        nc.register_instruction(ev, overwrite=True)
                        out.append(ev)
                    inst.sync_info = mybir.SyncInfo(
                        on_wait=waits[-max_waits:],
                        on_update=list(si.on_update or []),
                    )
                    changed = True
                out.append(inst)
            if changed:
                bb.instructions = out


# ---------------------------------------------------------------------------
# Host-side preprocessing
# ---------------------------------------------------------------------------
def _prep_core(x_b, offset_b, mask_b):
    """Build x_perm [NK, CIN, SLOTS] and oh [128, 128, NK, 128] for one batch."""
    off = offset_b.reshape(NK, 2, H, W).astype(np.float64)
    m = mask_b.reshape(NK, H * W).astype(np.float64)

    ky = (np.arange(NK) // KW).astype(np.float64) * DIL
    kx = (np.arange(NK) % KW).astype(np.float64) * DIL
    base_y = np.arange(H, dtype=np.float64) * STRIDE - PAD
    base_x = np.arange(W, dtype=np.float64) * STRIDE - PAD

    y = off[:, 0] + base_y[None, :, None] + ky[:, None, None]   # [NK, H, W]
    xp = off[:, 1] + base_x[None, None, :] + kx[:, None, None]

    iy0 = np.floor(y).astype(np.int64).reshape(NK, H * W)
    fy = (y - np.floor(y)).reshape(NK, H * W)
    ix0 = np.floor(xp).astype(np.int64).reshape(NK, H * W)
    fx = (xp - np.floor(xp)).reshape(NK, H * W)

    bi = iy0 + 1
    valid = (bi >= 0) & (bi <= 128)

    x_perm = np.zeros((NK, CIN, SLOTS), dtype=np.float32)
    # even output rows r=2m: the slot chunk [64r, 64r+128) IS V tile m -> one
    # K=128 matmul per tap, one-hot ohe[r//2].
    # odd rows straddle tiles m=(r-1)/2, m+1; HW matmuls must start at
    # partition 0, so use two full K=128 matmuls against each whole tile with
    # out-of-chunk rows zeroed: oho[(r-1)//2, 0] (vs tile m) and
    # oho[(r-1)//2, 1] (vs tile m+1).
    ohe = np.zeros((OHH // 2, 128, NK, OWW), dtype=np.float32)
    oho = np.zeros((OHH // 2, 2, 128, NK, OWW), dtype=np.float32)

    for k in range(NK):
        pxv = np.nonzero(valid[k])[0]
        order = pxv[np.argsort(bi[k, pxv], kind="stable")]
        bsort = bi[k, order]
        # rank within bucket
        start = np.searchsorted(bsort, np.arange(NBUCKET), side="left")
        rank = np.arange(len(order)) - start[bsort]
        fill = np.bincount(bsort, minlength=NBUCKET)
        if fill.max() > C - 2:
            raise RuntimeError(f"bucket overflow: max fill {fill.max()} > {C-2}")
        slot = bsort * C + rank
        x_perm[k][:, slot] = x_b[:, order]

        # corner dy=0 hits row r0=bi-1 with wy=1-fy; corner dy=1 hits row
        # r1=bi with wy=fy.  Local index within the target chunk's V tile(s):
        #   even r, bucket r+1 (dy=0): local 64+rank   | bucket r (dy=1): rank
        #   odd  r: part a = tile m local 64+rank (dy=1 corner, bucket r)
        #           part b = tile m+1 local rank (dy=0 corner, bucket r+1)
        for (r_arr, wy, is_dy0) in (
            (bsort - 1, 1.0 - fy[k, order], True),
            (bsort, fy[k, order], False),
        ):
            rok = (r_arr >= 0) & (r_arr <= OHH - 1)
            even = rok & (r_arr % 2 == 0)
            odd = rok & (r_arr % 2 == 1)
            for (col, wx) in (
                (ix0[k, order], 1.0 - fx[k, order]),
                (ix0[k, order] + 1, fx[k, order]),
            ):
                colok = (col >= 0) & (col <= OWW - 1)
                wgt = m[k, order] * wy * wx
                ce = even & colok
                co = odd & colok
                if is_dy0:
                    ohe[r_arr[ce] // 2, 64 + rank[ce], k, col[ce]] = wgt[ce]
                    oho[(r_arr[co] - 1) // 2, 1, rank[co], k, col[co]] = wgt[co]
                else:
                    ohe[r_arr[ce] // 2, rank[ce], k, col[ce]] = wgt[ce]
                    oho[(r_arr[co] - 1) // 2, 0, 64 + rank[co], k, col[co]] = wgt[co]

    # bias hook: all-ones one-hot row pointing at a pad V slot (rank 63 of
    # bucket r+1) that the device overwrites with the bias vector.
    ohe[:, 127, 0, :] = 1.0   # tile m partition 127 = bucket 2m+1 rank 63
    oho[:, 1, 63, 0, :] = 1.0  # tile m+1 partition 63 = bucket r+1 rank 63
    return x_perm.astype(BF16), ohe.astype(BF16), oho.astype(BF16)


def _prep_all(x, weight, offset, mask, bias):
    w_r = np.ascontiguousarray(
        weight.reshape(CIN, COUT, NK).transpose(0, 2, 1)
    ).astype(BF16)  # [CIN, NK, COUT]
    bias_rep = np.zeros((128, COUT), np.float32)
    bias_rep[63] = bias
    bias_rep[127] = bias
    bias_rep = bias_rep.astype(BF16)
    in_maps = []
    for b in range(B):
        x_perm, ohe, oho = _prep_core(x[b].reshape(CIN, H * W), offset[b], mask[b])
        in_maps.append({"xp": x_perm, "ohe": ohe, "oho": oho, "w": w_r, "br": bias_rep})
    return in_maps


# ---------------------------------------------------------------------------
# Device program
# ---------------------------------------------------------------------------
def build_nc(reps=1):
    nc = bass.Bass()
    xp = nc.dram_tensor("xp", [NK, CIN, SLOTS], mybir.dt.bfloat16, kind="ExternalInput")
    ohe = nc.dram_tensor("ohe", [OHH // 2, 128, NK, OWW], mybir.dt.bfloat16, kind="ExternalInput")
    oho = nc.dram_tensor("oho", [OHH // 2, 2, 128, NK, OWW], mybir.dt.bfloat16, kind="ExternalInput")
    wd = nc.dram_tensor("w", [CIN, NK, COUT], mybir.dt.bfloat16, kind="ExternalInput")
    brd = nc.dram_tensor("br", [128, COUT], mybir.dt.bfloat16, kind="ExternalInput")
    out = nc.dram_tensor("out", [OHH * OWW, COUT], mybir.dt.float32, kind="ExternalOutput")

    with tile.TileContext(nc) as tc:
        with tc.tile_pool(name="const", bufs=1) as cpool, \
             tc.tile_pool(name="xt", bufs=6) as xpool, \
             tc.tile_pool(name="v", bufs=2 * NK * TILES_PER_STRIP + 8) as vpool, \
             tc.tile_pool(name="oht", bufs=6) as ohpool, \
             tc.tile_pool(name="ot", bufs=6) as opool, \
             tc.tile_pool(name="pg", bufs=3, space="PSUM") as pgpool, \
             tc.tile_pool(name="po", bufs=3, space="PSUM") as popool:

            # resident weights [2][128, NK*COUT] and bias [2, COUT]
            wt = []
            for h in range(2):
                t = cpool.tile([128, NK * COUT], mybir.dt.bfloat16, tag=f"w{h}")
                nc.sync.dma_start(out=t[:], in_=wd[h * 128:(h + 1) * 128].rearrange("a b c -> a (b c)"))
                wt.append(t)
            bt = cpool.tile([128, COUT], mybir.dt.bfloat16, tag="bias")
            nc.sync.dma_start(out=bt[:], in_=brd[:])

            for rep in range(reps):
                for s in range(NSTRIP):
                    base_tile = NSTRIP * s  # first V tile index of this strip
                    # ---- GEMM phase: V tiles [base_tile, base_tile+9) per tap
                    vt = {}
                    for k in range(NK):
                        xts = []
                        for h in range(2):
                            t = xpool.tile([128, TILES_PER_STRIP * 128],
                                           mybir.dt.bfloat16, tag=f"x{h}")
                            nc.sync.dma_start(
                                out=t[:],
                                in_=xp[k, h * 128:(h + 1) * 128,
                                       base_tile * 128:(base_tile + TILES_PER_STRIP) * 128],
                            )
                            xts.append(t)
                        for c in range(TILES_PER_STRIP):
                            ps = pgpool.tile([128, COUT], mybir.dt.float32, tag="pg")
                            for h in range(2):
                                nc.tensor.matmul(
                                    out=ps[:],
                                    lhsT=xts[h][:, c * 128:(c + 1) * 128],
                                    rhs=wt[h][:, k * COUT:(k + 1) * COUT],
                                    start=(h == 0),
                                    stop=(h == 1),
                                )
                            v = vpool.tile([128, COUT], mybir.dt.bfloat16, tag="v")
                            if k == 0:
                                # fold bias into the pad slots (partitions
                                # 63/127 are always-zero bucket pad ranks)
                                nc.vector.tensor_add(out=v[:], in0=ps[:], in1=bt[:])
                            elif (k * TILES_PER_STRIP + c) % 2 == 0:
                                nc.vector.tensor_copy(out=v[:], in_=ps[:])
                            else:
                                nc.scalar.copy(out=v[:], in_=ps[:])
                            vt[(k, c)] = v

                    # ---- scatter phase: output rows of this strip
                    for j in range(ROWS_PER_STRIP):
                        r = s * ROWS_PER_STRIP + j
                        if j % 2 == 0:
                            oht = ohpool.tile([128, NK * OWW], mybir.dt.bfloat16, tag="ohe")
                            nc.sync.dma_start(
                                out=oht[:], in_=ohe[r // 2].rearrange("a b c -> a (b c)")
                            )
                            # (oh column offset, V tile local index)
                            mms = [(0, j // 2)]
                        else:
                            oht = ohpool.tile([128, 2 * NK * OWW], mybir.dt.bfloat16, tag="oho")
                            for t in range(2):
                                nc.sync.dma_start(
                                    out=oht[:, t * NK * OWW:(t + 1) * NK * OWW],
                                    in_=oho[(r - 1) // 2, t].rearrange("a b c -> a (b c)"),
                                )
                            mms = [(0, (j - 1) // 2), (NK * OWW, (j - 1) // 2 + 1)]
                        po = popool.tile([128, COUT], mybir.dt.float32, tag="po")
                        chain = [(k, off, cidx) for k in range(NK) for (off, cidx) in mms]
                        for i, (k, off, cidx) in enumerate(chain):
                            nc.tensor.matmul(
                                out=po[:],
                                lhsT=oht[:, off + k * OWW:off + (k + 1) * OWW],
                                rhs=vt[(k, cidx)][:],
                                start=(i == 0),
                                stop=(i == len(chain) - 1),
                            )
                        ot = opool.tile([128, COUT], mybir.dt.float32, tag="ot")
                        if j % 2 == 0:
                            nc.vector.tensor_copy(out=ot[:], in_=po[:])
                        else:
                            nc.scalar.copy(out=ot[:], in_=po[:])
                        nc.sync.dma_start(out=out[r * OWW:(r + 1) * OWW, :], in_=ot[:])
    _split_multi_waits(nc)
    return nc


class Runner:
    """Persistent multi-core executable: compile/load once, dispatch cheaply."""

    def __init__(self, reps=1):
        import jax
        from jax.sharding import Mesh, PartitionSpec
        from jax.experimental.shard_map import shard_map
        from concourse.bass2jax import (
            _bass_exec_p, install_neuronx_cc_hook, partition_id_tensor,
        )

        install_neuronx_cc_hook()
        nc = build_nc(reps)
        self.nc = nc
        in_names, out_names, out_avals, zero_outs = [], [], [], []
        pname = nc.partition_id_tensor.name if nc.partition_id_tensor else None
        for alloc in nc.m.functions[0].allocations:
            if not isinstance(alloc, mybir.MemoryLocationSet):
                continue
            name = alloc.memorylocations[0].name
            if alloc.kind == "ExternalInput":
                if name != pname:
                    in_names.append(name)
            elif alloc.kind == "ExternalOutput":
                shape = tuple(alloc.tensor_shape)
                dtype = mybir.dt.np(alloc.dtype)
                out_avals.append(jax.core.ShapedArray(shape, dtype))
                out_names.append(name)
                zero_outs.append(np.zeros((B * shape[0], *shape[1:]), dtype))
        self.in_names, self.out_names = in_names, out_names
        self.out_avals, self.zero_outs = out_avals, zero_outs
        n_params = len(in_names)
        all_in = in_names + out_names + ([pname] if pname else [])

        def _body(*args):
            operands = list(args)
            if pname:
                operands.append(partition_id_tensor())
            return tuple(_bass_exec_p.bind(
                *operands, out_avals=tuple(out_avals), in_names=tuple(all_in),
                out_names=tuple(out_names), lowering_input_output_aliases=(),
                sim_require_finite=True, sim_require_nnan=True, nc=nc))

        devices = jax.devices()[:B]
        mesh = Mesh(np.asarray(devices), ("core",))
        in_specs = (PartitionSpec("core"),) * (n_params + len(out_avals))
        out_specs = (PartitionSpec("core"),) * len(out_names)
        self._jit = jax.jit(
            shard_map(_body, mesh=mesh, in_specs=in_specs, out_specs=out_specs,
                      check_rep=False),
            donate_argnums=tuple(range(n_params, n_params + len(out_avals))),
            keep_unused=True,
        )
        self._jax = jax

    def concat_inputs(self, in_maps):
        return [np.concatenate([np.asarray(m[n]) for m in in_maps], axis=0)
                for n in self.in_names]

    def __call__(self, concat_in):
        outs = self._jit(*concat_in, *[z.copy() for z in self.zero_outs])
        self._jax.block_until_ready(outs)
        return [
            {name: np.asarray(outs[i]).reshape(B, *self.out_avals[i].shape)[c]
             for i, name in enumerate(self.out_names)}
            for c in range(B)
        ]


_RUNNERS = {}


def get_runner(reps=1):
    if reps not in _RUNNERS:
        _RUNNERS[reps] = Runner(reps)
    return _RUNNERS[reps]


def run_on_hw(in_maps, reps=1):
    r = get_runner(reps)
    return r(r.concat_inputs(in_maps))


def kernel(x, weight, offset, mask, bias):
    x = np.asarray(x, dtype=np.float32)
    weight = np.asarray(weight, dtype=np.float32)
    offset = np.asarray(offset, dtype=np.float32)
    mask = np.asarray(mask, dtype=np.float32)
    bias = np.asarray(bias, dtype=np.float32)

    in_maps = _prep_all(x, weight, offset, mask, bias)
    results = run_on_hw(in_maps, reps=1)
    out = np.empty((B, COUT, OHH, OWW), dtype=np.float32)
    for b in range(B):
        out[b] = results[b]["out"].T.reshape(COUT, OHH, OWW)
    return out
